# revision 1
# baseline (speedup 1.0000x reference)
"""DbrxExpertGLU (single-expert SwiGLU MLP) Trainium2 kernel.

  down = (silu(x @ w1.T) * (x @ v1.T)) @ w2
  x: [4096, 4096] f32, w1/v1/w2: [14336, 4096] f32 -> out [4096, 4096] f32

Strategy (8 NeuronCores, tensor-parallel over ffn dim per the expert-TP
hint): shard F=14336 into 8 x 1792. Each core computes gate/up/inter for
its F-shard and a partial down [4096, 4096]; the host sums the 8 fp32
partials (cheaper than an on-device all-reduce and off the HW critical
path).

On-device layout is activation-transposed ([feature, token]) so all three
matmuls chain with weights stationary and no transposes:
  gateT[f,t] = sum_h w1[f,h] x[t,h];  upT likewise
  interT     = sigmoid(gateT)*gateT*upT  (ACT+DVE, cast to bf16)
  downT[h,t] = sum_f w2[f,h] interT[f,t]
Matmuls run in bf16 (fp32 PSUM accumulation) -> PE at 1 cycle/row; the
whole kernel is PE-bound at ~98% of the bf16 roofline (~2.3 ms/core).
Host pre-casts/pre-tiles inputs so every DMA lands >=1KB-contiguous per
partition.
"""

import os
import subprocess
import sys
import tempfile
import time
from contextlib import ExitStack

import numpy as np
import ml_dtypes

import concourse.bass as bass
import concourse.mybir as mybir
import concourse.tile as tile
from concourse import bacc
from concourse.bass_utils import run_bass_kernel_spmd

BF16 = mybir.dt.bfloat16
F32 = mybir.dt.float32

T, H, F = 4096, 4096, 14336
N_CORES = 8
FS = F // N_CORES           # 1792 ffn rows per core
TC = 512                    # token chunk (= matmul moving dim)
NT, KB, FBN, HB = T // TC, H // 128, FS // 128, H // 128

_NC_CACHE = []


def _build():
    nc = bacc.Bacc("TRN2", target_bir_lowering=False, debug=False)

    xh = nc.dram_tensor("xh", [NT, KB, 128, TC], BF16, kind="ExternalInput").ap()
    w1h = nc.dram_tensor("w1h", [FBN, 128, KB, 128], BF16, kind="ExternalInput").ap()
    v1h = nc.dram_tensor("v1h", [FBN, 128, KB, 128], BF16, kind="ExternalInput").ap()
    w2h = nc.dram_tensor("w2h", [HB, 128, FBN, 128], BF16, kind="ExternalInput").ap()
    out = nc.dram_tensor("out", [H, T], F32, kind="ExternalOutput").ap()

    with tile.TileContext(nc) as tc, ExitStack() as ctx:
        xc_pool = ctx.enter_context(tc.tile_pool(name="xc", bufs=2))
        w1_pool = ctx.enter_context(tc.tile_pool(name="w1", bufs=3))
        v1_pool = ctx.enter_context(tc.tile_pool(name="v1", bufs=3))
        w2_pool = ctx.enter_context(tc.tile_pool(name="w2", bufs=3))
        inter_pool = ctx.enter_context(tc.tile_pool(name="inter", bufs=2))
        silu_pool = ctx.enter_context(tc.tile_pool(name="silu", bufs=3))
        out_pool = ctx.enter_context(tc.tile_pool(name="outp", bufs=4))
        pg_pool = ctx.enter_context(tc.tile_pool(name="pg", bufs=2, space="PSUM"))
        pu_pool = ctx.enter_context(tc.tile_pool(name="pu", bufs=2, space="PSUM"))
        pd_pool = ctx.enter_context(tc.tile_pool(name="pd", bufs=3, space="PSUM"))

        for tci in range(NT):
            # x chunk, free dim = (kb, t): rhs tiles for every h-block
            xc = xc_pool.tile([128, KB * TC], BF16)
            if tci == 0:
                # fine-grained first load on the otherwise-idle ACT HWDGE
                # ring (parallel to weight DMAs on SP) so the PE starts on
                # kb=0 ~13us sooner instead of waiting for the whole 4MB;
                # extra-fine leading slices, 4-kb steady slices
                bounds = [0, 2, 4] + list(range(8, KB + 1, 4))
                for k0, k1 in zip(bounds, bounds[1:]):
                    nc.scalar.dma_start(
                        out=xc[:, k0 * TC:k1 * TC].rearrange(
                            "p (kb t) -> p kb t", kb=k1 - k0
                        ),
                        in_=xh[tci, k0:k1].rearrange("kb p t -> p kb t"),
                    )
            else:
                nc.sync.dma_start(
                    out=xc[:].rearrange("p (kb t) -> p kb t", kb=KB),
                    in_=xh[tci].rearrange("kb p t -> p kb t"),
                )
            inter = inter_pool.tile([128, FBN * TC], BF16)

            # phase A: gateT/upT -> interT, one f-block (128 rows) at a time
            for fb in range(FBN):
                w1f = w1_pool.tile([128, KB * 128], BF16)
                if tci == 0 and fb == 0:
                    for k0 in range(0, KB, 8):
                        nc.sync.dma_start(
                            out=w1f[:, k0 * 128:(k0 + 8) * 128].rearrange(
                                "p (kb f) -> p kb f", kb=8
                            ),
                            in_=w1h[fb][:, k0:k0 + 8],
                        )
                else:
                    nc.sync.dma_start(
                        out=w1f[:].rearrange("p (kb f) -> p kb f", kb=KB), in_=w1h[fb]
                    )
                v1f = v1_pool.tile([128, KB * 128], BF16)
                nc.sync.dma_start(
                    out=v1f[:].rearrange("p (kb f) -> p kb f", kb=KB), in_=v1h[fb]
                )
                pg = pg_pool.tile([128, TC], F32)
                pu = pu_pool.tile([128, TC], F32)
                for kb in range(KB):
                    nc.tensor.matmul(
                        pg[:], w1f[:, bass.ts(kb, 128)], xc[:, bass.ts(kb, TC)],
                        start=(kb == 0), stop=(kb == KB - 1),
                    )
                for kb in range(KB):
                    nc.tensor.matmul(
                        pu[:], v1f[:, bass.ts(kb, 128)], xc[:, bass.ts(kb, TC)],
                        start=(kb == 0), stop=(kb == KB - 1),
                    )
                sg = silu_pool.tile([128, TC], F32)
                nc.scalar.activation(
                    sg[:], pg[:], mybir.ActivationFunctionType.Sigmoid
                )
                sl = silu_pool.tile([128, TC], F32)
                nc.vector.tensor_mul(sl[:], sg[:], pg[:])
                nc.vector.tensor_mul(inter[:, bass.ts(fb, TC)], sl[:], pu[:])

            # phase B: partial downT, one h-block at a time
            for hb in range(HB):
                w2t = w2_pool.tile([128, FBN * 128], BF16)
                nc.sync.dma_start(
                    out=w2t[:].rearrange("p (fb h) -> p fb h", fb=FBN), in_=w2h[hb]
                )
                # final output tile: two N=256 groups (same PE cycles) so the
                # first half's copy+DMA-out hides under the second half's
                # matmuls instead of dangling off the kernel tail
                last_tile = tci == NT - 1 and hb == HB - 1
                splits = (0, 256, 384, 512) if last_tile else (0, TC)
                for si in range(len(splits) - 1):
                    c0, c1 = splits[si], splits[si + 1]
                    pd = pd_pool.tile([128, c1 - c0], F32)
                    for fb in range(FBN):
                        nc.tensor.matmul(
                            pd[:], w2t[:, bass.ts(fb, 128)],
                            inter[:, fb * TC + c0:fb * TC + c1],
                            start=(fb == 0), stop=(fb == FBN - 1),
                        )
                    ob = out_pool.tile([128, c1 - c0], F32)
                    nc.scalar.copy(ob[:], pd[:])
                    nc.sync.dma_start(
                        out=out[hb * 128:(hb + 1) * 128,
                                tci * TC + c0:tci * TC + c1],
                        in_=ob[:],
                    )

    nc.compile()
    return nc


def _prep_inputs(x, w1, v1, w2):
    bf = ml_dtypes.bfloat16
    # x[t, h] -> xh[tc, kb, p(h%128), tt]
    xh = np.ascontiguousarray(
        x.astype(bf).reshape(NT, TC, KB, 128).transpose(0, 2, 3, 1)
    )
    in_maps = []
    for c in range(N_CORES):
        sl = slice(c * FS, (c + 1) * FS)
        w1s = w1[sl].astype(bf)
        v1s = v1[sl].astype(bf)
        w2s = w2[sl].astype(bf)
        in_maps.append({
            "xh": xh,
            # w1[f, h] -> [fb, p(h%128), kb, ff]
            "w1h": np.ascontiguousarray(
                w1s.reshape(FBN, 128, KB, 128).transpose(0, 3, 2, 1)
            ),
            "v1h": np.ascontiguousarray(
                v1s.reshape(FBN, 128, KB, 128).transpose(0, 3, 2, 1)
            ),
            # w2[f, h] -> [hb, p(f%128), fb, hh]
            "w2h": np.ascontiguousarray(
                w2s.reshape(FBN, 128, HB, 128).transpose(2, 1, 0, 3)
            ),
        })
    return in_maps


def _exec_once(in_maps):
    """One 8-core device execution; returns summed partial [H, T] f32."""
    if not _NC_CACHE:
        _NC_CACHE.append(_build())
    res = run_bass_kernel_spmd(_NC_CACHE[0], in_maps, list(range(N_CORES)))
    acc = res.results[0]["out"].astype(np.float32)
    for c in range(1, N_CORES):
        acc += res.results[c]["out"]
    if not np.isfinite(acc).all():
        raise FloatingPointError("non-finite output from device")
    return acc


def _exec_subprocess(in_maps):
    """Retry path: run the device execution in a fresh process (fresh axon
    client) in case this process's device session is poisoned."""
    base = "/dev/shm" if os.path.isdir("/dev/shm") else None
    with tempfile.TemporaryDirectory(dir=base) as d:
        np.save(os.path.join(d, "xh.npy"), in_maps[0]["xh"].view(np.uint16))
        for c, m in enumerate(in_maps):
            for k in ("w1h", "v1h", "w2h"):
                np.save(os.path.join(d, f"{k}_{c}.npy"), m[k].view(np.uint16))
        subprocess.run(
            [sys.executable, os.path.abspath(__file__), "--subproc", d],
            check=True, timeout=1200,
        )
        return np.load(os.path.join(d, "acc.npy"))


def _subproc_main(d):
    bf = ml_dtypes.bfloat16
    xh = np.load(os.path.join(d, "xh.npy")).view(bf)
    in_maps = []
    for c in range(N_CORES):
        m = {"xh": xh}
        for k in ("w1h", "v1h", "w2h"):
            m[k] = np.load(os.path.join(d, f"{k}_{c}.npy")).view(bf)
        in_maps.append(m)
    np.save(os.path.join(d, "acc.npy"), _exec_once(in_maps))


def kernel(x, expert_w1, expert_v1, expert_w2):
    x = np.asarray(x, dtype=np.float32)
    expert_w1 = np.asarray(expert_w1, dtype=np.float32)
    expert_v1 = np.asarray(expert_v1, dtype=np.float32)
    expert_w2 = np.asarray(expert_w2, dtype=np.float32)
    assert x.shape == (T, H) and expert_w1.shape == (F, H)

    in_maps = _prep_inputs(x, expert_w1, expert_v1, expert_w2)

    acc = None
    last_err = None
    for attempt in range(4):
        try:
            if attempt < 2:
                acc = _exec_once(in_maps)
            else:
                acc = _exec_subprocess(in_maps)
            break
        except Exception as e:  # transient device/tunnel errors: retry
            last_err = e
            time.sleep(3.0)
    if acc is None:
        raise last_err
    return np.ascontiguousarray(acc.T)  # [h, t] -> [t, h]


if __name__ == "__main__" and len(sys.argv) == 3 and sys.argv[1] == "--subproc":
    _subproc_main(sys.argv[2])



# revision 4
# speedup vs baseline: 1.2934x; 1.2934x over previous
"""DbrxExpertGLU (single-expert SwiGLU MLP) Trainium2 kernel.

  down = (silu(x @ w1.T) * (x @ v1.T)) @ w2
  x: [4096, 4096] f32, w1/v1/w2: [14336, 4096] f32 -> out [4096, 4096] f32

Strategy (8 NeuronCores, tensor-parallel over ffn dim per the expert-TP
hint): shard F=14336 into 8 x 1792. Each core computes gate/up/inter for
its F-shard and a partial down [4096, 4096]; the host sums the 8 partials.

All three matmuls run in fp8 e4m3 DoubleRow mode (double-pumped PE rows:
each instruction contracts 256 rows at 0.5 cycles per output element).
To stay within the accuracy budget each operand is Dekker-split into
hi + lo e4m3 tensors and the three significant cross products
(hi*hi + hi*lo + lo*hi) are accumulated in PSUM, i.e. 3 double-pumped
passes replace 2 bf16-rate passes per 256 contraction rows -> 0.75x the
bf16 PE time with ~3e-3 relative error. Weights/x are split on the host
(with power-of-2 pre-scales to keep values in e4m3's normal range; the
scales are divided back out inside ACT ops). The SwiGLU intermediate is
split on-device (ACT copy for hi, one DVE scalar_tensor_tensor for lo).

Token chunks of 512 are processed in pairs sharing one weight pass so
weight DMA traffic halves; partial outputs return as bf16.
"""

import os
import subprocess
import sys
import tempfile
import time
from contextlib import ExitStack

import numpy as np
import ml_dtypes

import concourse.bass as bass
import concourse.mybir as mybir
import concourse.tile as tile
from concourse import bacc
from concourse.bass_utils import run_bass_kernel_spmd

BF16 = mybir.dt.bfloat16
F32 = mybir.dt.float32
FP8 = mybir.dt.float8e4
E4 = ml_dtypes.float8_e4m3
DR = mybir.MatmulPerfMode.DoubleRow
AF = mybir.ActivationFunctionType
ALU = mybir.AluOpType

T, H, F = 4096, 4096, 14336
N_CORES = 8
FS = F // N_CORES            # 1792 ffn rows per core
TC = 512                     # token chunk (= matmul moving dim)
NT = T // TC                 # 8 token chunks, processed in 4 pairs
C = H // 256                 # 16 k-chunks of 256 for gate/up contraction
FBN = FS // 128              # 14 f-blocks per core
JN = FS // 256               # 7 k-chunks of 256 for down contraction
HB = H // 128                # 32 h-blocks

# power-of-2 pre-scales keeping every fp8 operand in e4m3's normal range
SX, S1, SV, S2 = 4.0, 64.0, 64.0, 64.0
SIG_SCALE = 1.0 / (SX * S1)          # sigmoid arg: true gate
CAST_SCALE = 1.0 / (SX * SX * S1 * SV)  # raw psum product -> true inter
OUT_SCALE = 1.0 / S2                 # down psum -> true partial

_NC_CACHE = []


def _build():
    nc = bacc.Bacc("TRN2", target_bir_lowering=False, debug=False)

    xh_d = nc.dram_tensor("xh", [NT, 128, C, 2, TC], FP8, kind="ExternalInput").ap()
    xl_d = nc.dram_tensor("xl", [NT, 128, C, 2, TC], FP8, kind="ExternalInput").ap()
    w1h_d = nc.dram_tensor("w1h", [FBN, 128, C, 2, 128], FP8, kind="ExternalInput").ap()
    w1l_d = nc.dram_tensor("w1l", [FBN, 128, C, 2, 128], FP8, kind="ExternalInput").ap()
    v1h_d = nc.dram_tensor("v1h", [FBN, 128, C, 2, 128], FP8, kind="ExternalInput").ap()
    v1l_d = nc.dram_tensor("v1l", [FBN, 128, C, 2, 128], FP8, kind="ExternalInput").ap()
    w2h_d = nc.dram_tensor("w2h", [HB, 128, JN, 2, 128], FP8, kind="ExternalInput").ap()
    w2l_d = nc.dram_tensor("w2l", [HB, 128, JN, 2, 128], FP8, kind="ExternalInput").ap()
    out_d = nc.dram_tensor("out", [H, T], BF16, kind="ExternalOutput").ap()

    with tile.TileContext(nc) as tc, ExitStack() as ctx:
        xh_pool = ctx.enter_context(tc.tile_pool(name="xh", bufs=3))
        xl_pool = ctx.enter_context(tc.tile_pool(name="xl", bufs=3))
        w1h_pool = ctx.enter_context(tc.tile_pool(name="w1h", bufs=2))
        w1l_pool = ctx.enter_context(tc.tile_pool(name="w1l", bufs=2))
        v1h_pool = ctx.enter_context(tc.tile_pool(name="v1h", bufs=2))
        v1l_pool = ctx.enter_context(tc.tile_pool(name="v1l", bufs=2))
        w2h_pool = ctx.enter_context(tc.tile_pool(name="w2h", bufs=2))
        w2l_pool = ctx.enter_context(tc.tile_pool(name="w2l", bufs=2))
        ihi_pool = ctx.enter_context(tc.tile_pool(name="ihi", bufs=2))
        ilo_pool = ctx.enter_context(tc.tile_pool(name="ilo", bufs=2))
        sg_pool = ctx.enter_context(tc.tile_pool(name="sg", bufs=2))
        sl_pool = ctx.enter_context(tc.tile_pool(name="sl", bufs=2))
        raw_pool = ctx.enter_context(tc.tile_pool(name="raw", bufs=2))
        out_pool = ctx.enter_context(tc.tile_pool(name="outp", bufs=4))
        pg_pool = ctx.enter_context(tc.tile_pool(name="pg", bufs=2, space="PSUM"))
        pu_pool = ctx.enter_context(tc.tile_pool(name="pu", bufs=2, space="PSUM"))
        pd_pool = ctx.enter_context(tc.tile_pool(name="pd", bufs=3, space="PSUM"))

        def load_x(tt, fine):
            xh_t = xh_pool.tile([128, C * 2 * TC], FP8, name="xht")
            xl_t = xl_pool.tile([128, C * 2 * TC], FP8, name="xlt")
            for tl_, src in ((xh_t, xh_d), (xl_t, xl_d)):
                dst = tl_[:].rearrange("p (c i t) -> p c i t", c=C, i=2)
                if fine:
                    # fine first-load: PE can start on chunk c=0 early
                    bounds = [0, 1, 2, 4] + list(range(8, C + 1, 4))
                    for c0, c1 in zip(bounds, bounds[1:]):
                        nc.scalar.dma_start(out=dst[:, c0:c1],
                                            in_=src[tt, :, c0:c1])
                else:
                    nc.scalar.dma_start(out=dst, in_=src[tt])
            return xh_t, xl_t

        def load_w1(fb, fine):
            tiles = []
            for pool, src in ((w1h_pool, w1h_d), (w1l_pool, w1l_d),
                              (v1h_pool, v1h_d), (v1l_pool, v1l_d)):
                t_ = pool.tile([128, C * 2 * 128], FP8, name="wt")
                dst = t_[:].rearrange("p (c i m) -> p c i m", c=C, i=2)
                if fine:
                    for c0 in range(0, C, 4):
                        nc.sync.dma_start(out=dst[:, c0:c0 + 4],
                                          in_=src[fb, :, c0:c0 + 4])
                else:
                    nc.sync.dma_start(out=dst, in_=src[fb])
                tiles.append(t_)
            return tiles

        for pr in range(NT // 2):
            t0, t1 = 2 * pr, 2 * pr + 1
            if pr == 0:
                xt = {t0: load_x(t0, True), t1: load_x(t1, False)}
            else:
                # tiles were prefetched during the previous pair's phase B
                xt = {t0: xt_next[t0], t1: xt_next[t1]}

            ihi = {tt: ihi_pool.tile([128, FBN * TC], FP8, name="ihit")
                   for tt in (t0, t1)}
            ilo = {tt: ilo_pool.tile([128, FBN * TC], FP8, name="ilot")
                   for tt in (t0, t1)}

            # phase A: gate/up -> inter (hi/lo e4m3), per 128-row f-block
            for fb in range(FBN):
                w1h_t, w1l_t, v1h_t, v1l_t = load_w1(fb, pr == 0 and fb == 0)
                w1hv = w1h_t[:].rearrange("p (c i m) -> p c i m", c=C, i=2)
                w1lv = w1l_t[:].rearrange("p (c i m) -> p c i m", c=C, i=2)
                v1hv = v1h_t[:].rearrange("p (c i m) -> p c i m", c=C, i=2)
                v1lv = v1l_t[:].rearrange("p (c i m) -> p c i m", c=C, i=2)
                for tt in (t0, t1):
                    xhv = xt[tt][0][:].rearrange("p (c i t) -> p c i t", c=C, i=2)
                    xlv = xt[tt][1][:].rearrange("p (c i t) -> p c i t", c=C, i=2)
                    pg = pg_pool.tile([128, TC], F32)
                    pu = pu_pool.tile([128, TC], F32)
                    k = 0
                    for c in range(C):
                        for wt, xv in ((w1hv, xhv), (w1hv, xlv), (w1lv, xhv)):
                            nc.tensor.matmul(pg[:], wt[:, c], xv[:, c],
                                             start=(k == 0), stop=(k == 3 * C - 1),
                                             perf_mode=DR)
                            k += 1
                    k = 0
                    for c in range(C):
                        for wt, xv in ((v1hv, xhv), (v1hv, xlv), (v1lv, xhv)):
                            nc.tensor.matmul(pu[:], wt[:, c], xv[:, c],
                                             start=(k == 0), stop=(k == 3 * C - 1),
                                             perf_mode=DR)
                            k += 1
                    sg = sg_pool.tile([128, TC], F32)
                    nc.scalar.activation(sg[:], pg[:], AF.Sigmoid, scale=SIG_SCALE)
                    sl = sl_pool.tile([128, TC], F32)
                    nc.vector.tensor_mul(sl[:], sg[:], pg[:])
                    raw = raw_pool.tile([128, TC], F32)
                    nc.vector.tensor_mul(raw[:], sl[:], pu[:])
                    hi_sl = ihi[tt][:, bass.ts(fb, TC)]
                    nc.scalar.activation(hi_sl, raw[:], AF.Copy, scale=CAST_SCALE)
                    nc.vector.scalar_tensor_tensor(
                        ilo[tt][:, bass.ts(fb, TC)], raw[:], CAST_SCALE, hi_sl,
                        op0=ALU.mult, op1=ALU.subtract)

            # prefetch next pair's x during phase B
            if pr < NT // 2 - 1:
                xt_next = {2 * pr + 2: load_x(2 * pr + 2, False),
                           2 * pr + 3: load_x(2 * pr + 3, False)}

            # phase B: partial down, per 128-row h-block
            for hb in range(HB):
                w2h_t = w2h_pool.tile([128, JN * 2 * 128], FP8)
                w2l_t = w2l_pool.tile([128, JN * 2 * 128], FP8)
                nc.sync.dma_start(
                    out=w2h_t[:].rearrange("p (j i m) -> p j i m", j=JN, i=2),
                    in_=w2h_d[hb])
                nc.sync.dma_start(
                    out=w2l_t[:].rearrange("p (j i m) -> p j i m", j=JN, i=2),
                    in_=w2l_d[hb])
                w2hv = w2h_t[:].rearrange("p (j i m) -> p j i m", j=JN, i=2)
                w2lv = w2l_t[:].rearrange("p (j i m) -> p j i m", j=JN, i=2)
                for tt in (t0, t1):
                    ihv = ihi[tt][:].rearrange("p (fb t) -> p fb t", fb=FBN)
                    ilv = ilo[tt][:].rearrange("p (fb t) -> p fb t", fb=FBN)
                    pd = pd_pool.tile([128, TC], F32)
                    k = 0
                    for j in range(JN):
                        for wt, iv in ((w2hv, ihv), (w2hv, ilv), (w2lv, ihv)):
                            nc.tensor.matmul(pd[:], wt[:, j],
                                             iv[:, 2 * j:2 * j + 2],
                                             start=(k == 0), stop=(k == 3 * JN - 1),
                                             perf_mode=DR)
                            k += 1
                    ob = out_pool.tile([128, TC], BF16)
                    nc.scalar.activation(ob[:], pd[:], AF.Copy, scale=OUT_SCALE)
                    nc.sync.dma_start(
                        out=out_d[hb * 128:(hb + 1) * 128,
                                  tt * TC:(tt + 1) * TC],
                        in_=ob[:])

    nc.compile()
    return nc


def _split(a):
    hi = a.astype(E4)
    lo = (a - hi.astype(np.float32)).astype(E4)
    return np.ascontiguousarray(hi), np.ascontiguousarray(lo)


def _prep_inputs(x, w1, v1, w2):
    # x[t, h]*SX -> [tci, p(h%128), c(h//256), i((h%256)//128), tt]
    x4 = (x * SX).reshape(NT, TC, C, 2, 128).transpose(0, 4, 2, 3, 1)
    xh, xl = _split(np.ascontiguousarray(x4, dtype=np.float32))
    in_maps = []
    for cid in range(N_CORES):
        sl_ = slice(cid * FS, (cid + 1) * FS)
        # w[f, h]*S -> [fb, p(h%128), c, i, m(f%128)]
        w1s = (w1[sl_] * S1).reshape(FBN, 128, C, 2, 128).transpose(0, 4, 2, 3, 1)
        v1s = (v1[sl_] * SV).reshape(FBN, 128, C, 2, 128).transpose(0, 4, 2, 3, 1)
        # w2[f, h]*S2 -> [hb, p(f%128), j(f//256), i((f%256)//128), m(h%128)]
        w2s = (w2[sl_] * S2).reshape(JN, 2, 128, HB, 128).transpose(3, 2, 0, 1, 4)
        w1h, w1l = _split(np.ascontiguousarray(w1s, dtype=np.float32))
        v1h, v1l = _split(np.ascontiguousarray(v1s, dtype=np.float32))
        w2h, w2l = _split(np.ascontiguousarray(w2s, dtype=np.float32))
        in_maps.append({
            "xh": xh, "xl": xl,
            "w1h": w1h, "w1l": w1l, "v1h": v1h, "v1l": v1l,
            "w2h": w2h, "w2l": w2l,
        })
    return in_maps


def _exec_once(in_maps):
    """One 8-core device execution; returns summed partial [H, T] f32."""
    if not _NC_CACHE:
        _NC_CACHE.append(_build())
    res = run_bass_kernel_spmd(_NC_CACHE[0], in_maps, list(range(N_CORES)))
    acc = res.results[0]["out"].astype(np.float32)
    for c in range(1, N_CORES):
        acc += res.results[c]["out"].astype(np.float32)
    if not np.isfinite(acc).all():
        raise FloatingPointError("non-finite output from device")
    return acc


_IN_KEYS = ("xh", "xl", "w1h", "w1l", "v1h", "v1l", "w2h", "w2l")


def _exec_subprocess(in_maps):
    """Retry path: run the device execution in a fresh process (fresh axon
    client) in case this process's device session is poisoned."""
    base = "/dev/shm" if os.path.isdir("/dev/shm") else None
    with tempfile.TemporaryDirectory(dir=base) as d:
        np.save(os.path.join(d, "xh.npy"), in_maps[0]["xh"].view(np.uint8))
        np.save(os.path.join(d, "xl.npy"), in_maps[0]["xl"].view(np.uint8))
        for c, m in enumerate(in_maps):
            for k in _IN_KEYS[2:]:
                np.save(os.path.join(d, f"{k}_{c}.npy"), m[k].view(np.uint8))
        subprocess.run(
            [sys.executable, os.path.abspath(__file__), "--subproc", d],
            check=True, timeout=1800,
        )
        return np.load(os.path.join(d, "acc.npy"))


def _subproc_main(d):
    xh = np.load(os.path.join(d, "xh.npy")).view(E4)
    xl = np.load(os.path.join(d, "xl.npy")).view(E4)
    in_maps = []
    for c in range(N_CORES):
        m = {"xh": xh, "xl": xl}
        for k in _IN_KEYS[2:]:
            m[k] = np.load(os.path.join(d, f"{k}_{c}.npy")).view(E4)
        in_maps.append(m)
    np.save(os.path.join(d, "acc.npy"), _exec_once(in_maps))


def kernel(x, expert_w1, expert_v1, expert_w2):
    x = np.asarray(x, dtype=np.float32)
    expert_w1 = np.asarray(expert_w1, dtype=np.float32)
    expert_v1 = np.asarray(expert_v1, dtype=np.float32)
    expert_w2 = np.asarray(expert_w2, dtype=np.float32)
    assert x.shape == (T, H) and expert_w1.shape == (F, H)

    in_maps = _prep_inputs(x, expert_w1, expert_v1, expert_w2)

    acc = None
    last_err = None
    for attempt in range(4):
        try:
            if attempt < 2:
                acc = _exec_once(in_maps)
            else:
                acc = _exec_subprocess(in_maps)
            break
        except Exception as e:  # transient device/tunnel errors: retry
            last_err = e
            time.sleep(3.0)
    if acc is None:
        raise last_err
    return np.ascontiguousarray(acc.T)  # [h, t] -> [t, h]


if __name__ == "__main__" and len(sys.argv) == 3 and sys.argv[1] == "--subproc":
    _subproc_main(sys.argv[2])
